# revision 21
# baseline (speedup 1.0000x reference)
"""Fused attention kernel for Trainium2, SPMD over 8 NeuronCores.

Problem: nn_Attention_2808908611625
  q = primary @ Wq + bq;  k = ctx @ Wk + bk;  v = ctx @ Wv + bv
  out = softmax(q k^T / sqrt(1024) - 1e9 * mask) @ v

Sharding: core c handles batch b = c//2, query-row half h = c%2
  (1024 query rows per core, full K/V context of its batch; K/V projection
  split across the core pair and exchanged with a pair AllGather).

Precision plan (PE matmul cycles are the roofline; measured on HW):
  * Projections and PV run in bf16 (1 cyc/output-row; fp8 DoubleRow's ~2x
    only helps if ONE term suffices -- residual-split needs 3 terms and
    loses, plain-fp8 x/W or P/V fails the 2e-2 gate at ~3e-2).
  * S^T = k q runs in PLAIN fp8 DoubleRow at free-dim 512 (the profitable
    regime: 2 contraction k-tiles per instruction, measured ~1.9x bf16).
    q, k are quantized to fp8e4 at their projection PSUM evictions (free).
    This is the dominant error term: ~1.2e-2 output rel err, inside the
    gate, and S is the only matmul where one fp8 term is enough.

Layout/schedule:
  * Host pre-transposes everything so every DMA lands contraction-major;
    no TensorE transposes. ctx/Wk input DMAs are chunked (dc-pairs) and
    spread over the SP + ACT queues so the first K-proj matmul starts
    ~2us in instead of waiting for whole tensors.
  * S is computed TRANSPOSED: S^T[kv, q] = sum_m kT.T @ qT, so the exp
    eviction writes P^T in exactly the [kv, q] layout PV needs stationary.
  * Mask is applied MULTIPLICATIVELY: host ships keep = 1-mask (bf16);
    ACT evicts P^T = exp(S^T/32) (PSUM->SBUF bf16), then one in-place DVE
    multiply by keep zeroes the masked entries (exact; 16-bit DVE 2x).
    No max-subtraction: |S/32| <= ~2.5 so exp <= ~12, bf16-safe. The whole
    keep matrix is prefetched during phase 1 as one 2MB DMA.
  * The k pair-exchange (write + AllGather + readback) is issued before
    the v exchange on the gpsimd queue so kT is resident long before the
    S matmuls (no phase-boundary PE gap). Biases fold into PSUM evictions
    (ACT Identity + bias for k and half of q, DVE tensor_scalar for the
    other half, DVE scalar_tensor_tensor for v's per-column bv).
  * V carries a 1025th column of ones; PV's 1025 output columns split into
    PSUM groups of 341/342/342 (rowsum group first) so the softmax row-sum
    falls out of the ones column, evicted with a per-partition 1/rowsum
    ACT scale. Output bf16, host upcasts.
"""

import numpy as np
import ml_dtypes

import concourse.bass as bass
import concourse.mybir as mybir
import concourse.tile as tile
from concourse import bacc, bass_utils

BF = mybir.dt.bfloat16
F32 = mybir.dt.float32
FP8 = mybir.dt.float8e4
AF = mybir.ActivationFunctionType
ALU = mybir.AluOpType
DR = mybir.MatmulPerfMode.DoubleRow

B, LQ, LKV, D = 4, 2048, 2048, 1024
P = 128
LQ_LOC = (B * LQ) // 8  # 1024 query rows per core
DC = D // P             # 8 contraction chunks
M = D // P              # 8 attn-dim chunks
NT = 512                # psum tile width
HKV = LKV // 2          # per-core K/V rows (pair-sharded)
LTH = HKV // NT         # 2 own kv column tiles (K^T layout)
LCH = HKV // P          # 8 own kv row chunks (V layout)
LC = LKV // P           # 16 kv chunks total
QT = LQ_LOC // P        # 8 q row tiles per core
QH = LQ_LOC // NT       # 2 q halves for S^T
PVG = [(684, 341), (0, 342), (342, 342)]  # PV psum groups; rowsum group first
VW = D + 1              # v width incl. ones col at D

UNROLL_REPS = False
STUB_CC = False   # force the local-DMA collective stub even at reps=1 (sim)
S_FP8 = True      # S^T matmuls in fp8 DoubleRow (else bf16)
ADD_MASK = False  # additive -960*mask on PSUM (else multiplicative keep)
LOADS_BASELINE = True  # monolithic loads, all on the SP queue
CHUNK_SYNC = True  # chunked cT/wk, everything on the SP queue (overrides)


def build_nc(reps: int = 1):
    nc = bacc.Bacc("TRN2", num_swdge_queues=4, num_devices=8)

    pT_d = nc.dram_tensor("primaryT", (D, LQ_LOC), BF, kind="ExternalInput")
    cT_d = nc.dram_tensor("contextT", (D, HKV), BF, kind="ExternalInput")
    keepT_d = nc.dram_tensor("keepT", (LKV, LQ_LOC), BF, kind="ExternalInput")
    wq_d = nc.dram_tensor("Wq", (D, D), BF, kind="ExternalInput")
    wk_d = nc.dram_tensor("Wk", (D, D), BF, kind="ExternalInput")
    wv_d = nc.dram_tensor("Wv", (D, D), BF, kind="ExternalInput")
    bq_d = nc.dram_tensor("bq", (D,), F32, kind="ExternalInput")
    bk_d = nc.dram_tensor("bk", (D,), F32, kind="ExternalInput")
    bv_d = nc.dram_tensor("bv", (D,), BF, kind="ExternalInput")
    out_d = nc.dram_tensor("out", (LQ_LOC, D), BF, kind="ExternalOutput")

    with tile.TileContext(nc) as tc:
        with (
            tc.tile_pool(name="const", bufs=1) as const,
            tc.tile_pool(name="persist", bufs=1) as persist,
            tc.tile_pool(name="dram", bufs=1, space="DRAM") as dram,
            tc.tile_pool(name="mmps", bufs=3, space="PSUM") as mmps,
            tc.tile_pool(name="sps", bufs=3, space="PSUM") as sps,
            tc.tile_pool(name="avps", bufs=2, space="PSUM") as avps,
        ):
            # biases: b*_sb[p, m] = b[m*128 + p] (descriptor-heavy; ACT queue)
            bq_sb = const.tile([P, M], F32)
            bk_sb = const.tile([P, M], F32)
            with nc.allow_non_contiguous_dma(reason="tiny bias vectors"):
                nc.scalar.dma_start(bq_sb, bq_d[:].rearrange("(m p) -> p m", p=P))
                nc.scalar.dma_start(bk_sb, bk_d[:].rearrange("(m p) -> p m", p=P))

            # bv broadcast to all partitions: ones[1,128].T @ bv[1, D]
            bv_row = const.tile([1, D], BF)
            nc.scalar.dma_start(bv_row, bv_d[:].rearrange("(one n) -> one n", one=1))
            ones_row = const.tile([1, P], BF)
            nc.vector.memset(ones_row, 1.0)
            bv_bcast = const.tile([P, D], F32)

            qkd = FP8 if S_FP8 else BF
            qT = persist.tile([P, M, LQ_LOC], qkd)   # q^T   [dattn, lq]
            kT = persist.tile([P, M, LKV], qkd)      # k^T   [dattn, lkv]
            v_sb = persist.tile([P, LC, VW], BF)     # v+bv  [lkv, dout | 1]
            keep_sb = persist.tile([P, LC, LQ_LOC], BF)  # 1-mask [kv, lq]

            # pair exchange buffers (AllGather within core pairs). Own halves
            # are evicted in place into kT/v_sb; the gathered copy is read
            # back in global kv order.
            k_in = dram.tile([M, P, HKV], qkd, name="k_in")
            k_out = dram.tile([2, M, P, HKV], qkd, name="k_out")
            v_in = dram.tile([LCH, P, D], BF, name="v_in")
            v_out = dram.tile([2, LCH, P, D], BF, name="v_out")
            RG = [[0, 1], [2, 3], [4, 5], [6, 7]]

            # bv broadcast is constant across reps -- compute once
            for n in range(D // NT):
                ps = mmps.tile([P, NT], F32, tag="mm", name="ps")
                nc.tensor.matmul(
                    ps, ones_row, bv_row[:, bass.ts(n, NT)],
                    start=True, stop=True,
                )
                nc.scalar.activation(bv_bcast[:, bass.ts(n, NT)], ps, AF.Copy)

            collective_in_body = (reps == 1 or UNROLL_REPS) and not STUB_CC
            loop_ctx = None
            if reps > 1 and not UNROLL_REPS:
                loop_ctx = tc.For_i(0, reps, 1)
                loop_ctx.__enter__()

            for _rep in range(reps if UNROLL_REPS else 1):
              # ---- phase 1: loads + K/V/Q projections + pair exchange ----
              with (
                  tc.tile_pool(name="w", bufs=1) as wp,
                  tc.tile_pool(name="xT", bufs=1) as xtp,
              ):
                  wq_sb = wp.tile([P, DC, D], BF)
                  wk_sb = wp.tile([P, DC, D], BF)
                  wv_sb = wp.tile([P, DC, D], BF)
                  pT = xtp.tile([P, DC, LQ_LOC], BF)  # primary^T [din, lq]
                  cT = xtp.tile([P, DC, HKV], BF)     # ctx^T [din, own half]

                  # SP queue: ctx + Wk interleaved in dc-pair chunks (K-proj
                  # starts after ~1MB), then primary. ACT queue: Wv, Wq, keep.
                  def chunk_load(eng, t, d, dp):
                      eng.dma_start(
                          t[:, 2 * dp : 2 * dp + 2, :],
                          d[2 * dp * P : (2 * dp + 2) * P, :].rearrange(
                              "(dc p) n -> p dc n", p=P
                          ),
                      )

                  if CHUNK_SYNC:
                      for dp in range(DC // 2):
                          chunk_load(nc.sync, cT, cT_d, dp)
                          chunk_load(nc.sync, wk_sb, wk_d, dp)
                      for t, d in ((wv_sb, wv_d), (pT, pT_d), (wq_sb, wq_d)):
                          nc.sync.dma_start(
                              t, d[:].rearrange("(dc p) n -> p dc n", p=P)
                          )
                      nc.sync.dma_start(
                          keep_sb,
                          keepT_d[:].rearrange("(lc p) n -> p lc n", p=P),
                      )
                  elif LOADS_BASELINE:
                      nc.sync.dma_start(
                          cT, cT_d[:].rearrange("(dc p) n -> p dc n", p=P)
                      )
                      for h in range(2):
                          HW2 = D // 2
                          nc.sync.dma_start(
                              wk_sb[:, :, h * HW2 : (h + 1) * HW2],
                              wk_d[:, h * HW2 : (h + 1) * HW2].rearrange(
                                  "(dc p) n -> p dc n", p=P
                              ),
                          )
                      nc.sync.dma_start(
                          wv_sb, wv_d[:].rearrange("(dc p) n -> p dc n", p=P)
                      )
                      nc.sync.dma_start(
                          pT, pT_d[:].rearrange("(dc p) n -> p dc n", p=P)
                      )
                      nc.sync.dma_start(
                          wq_sb, wq_d[:].rearrange("(dc p) n -> p dc n", p=P)
                      )
                      nc.sync.dma_start(
                          keep_sb,
                          keepT_d[:].rearrange("(lc p) n -> p lc n", p=P),
                      )
                  else:
                      for dp in range(DC // 2):
                          chunk_load(nc.sync, cT, cT_d, dp)
                          chunk_load(nc.sync, wk_sb, wk_d, dp)
                      nc.scalar.dma_start(
                          wv_sb, wv_d[:].rearrange("(dc p) n -> p dc n", p=P)
                      )
                      nc.sync.dma_start(
                          pT, pT_d[:].rearrange("(dc p) n -> p dc n", p=P)
                      )
                      nc.scalar.dma_start(
                          wq_sb, wq_d[:].rearrange("(dc p) n -> p dc n", p=P)
                      )
                      nc.scalar.dma_start(
                          keep_sb,
                          keepT_d[:].rearrange("(lc p) n -> p lc n", p=P),
                      )

                  # K^T own half -> kT half-0 slot (ACT folds bk, fp8 out);
                  # the post-collective readback rewrites kT in global order.
                  for l in range(LTH):
                      for m in range(M):
                          ps = mmps.tile([P, NT], F32, tag="mm", name="ps")
                          for dc in range(DC):
                              nc.tensor.matmul(
                                  ps,
                                  wk_sb[:, dc, bass.ts(m, P)],
                                  cT[:, dc, bass.ts(l, NT)],
                                  start=(dc == 0), stop=(dc == DC - 1),
                              )
                          nc.scalar.activation(
                              kT[:, m, bass.ts(l, NT)], ps, AF.Identity,
                              bias=bk_sb[:, m : m + 1],
                          )
                  # full k exchange chain first: kT must be resident for S
                  nc.gpsimd.dma_start(
                      k_in[:].rearrange("m p h -> p m h"), kT[:, :, 0:HKV]
                  )
                  if collective_in_body:
                      nc.gpsimd.collective_compute(
                          "AllGather", ALU.bypass, replica_groups=RG,
                          ins=[k_in[:]], outs=[k_out[:]],
                      )
                  else:  # timing stub: same bytes moved, no cross-core sync
                      for r in range(2):
                          nc.gpsimd.dma_start(
                              k_out[r].rearrange("m p h -> p m h"),
                              kT[:, :, 0:HKV],
                          )
                  for r in range(2):
                      nc.gpsimd.dma_start(
                          kT[:, :, r * HKV : (r + 1) * HKV],
                          k_out[r].rearrange("m p h -> p m h"),
                      )

                  # V own half (+bv via one DVE stt) -> v_sb chunk-0 slot
                  for lc in range(LCH):
                      for n in range(D // NT):
                          ps = mmps.tile([P, NT], F32, tag="mm", name="ps")
                          for dc in range(DC):
                              nc.tensor.matmul(
                                  ps,
                                  cT[:, dc, bass.ts(lc, P)],
                                  wv_sb[:, dc, bass.ts(n, NT)],
                                  start=(dc == 0), stop=(dc == DC - 1),
                              )
                          nc.vector.scalar_tensor_tensor(
                              v_sb[:, lc, bass.ts(n, NT)], ps, 1.0,
                              bv_bcast[:, bass.ts(n, NT)],
                              op0=ALU.mult, op1=ALU.add,
                          )
                  nc.gpsimd.dma_start(
                      v_in[:].rearrange("c p n -> p c n"), v_sb[:, 0:LCH, 0:D]
                  )
                  if collective_in_body:
                      nc.gpsimd.collective_compute(
                          "AllGather", ALU.bypass, replica_groups=RG,
                          ins=[v_in[:]], outs=[v_out[:]],
                      )
                  else:  # timing stub
                      for r in range(2):
                          nc.gpsimd.dma_start(
                              v_out[r].rearrange("c p n -> p c n"),
                              v_sb[:, 0:LCH, 0:D],
                          )
                  for r in range(2):
                      nc.gpsimd.dma_start(
                          v_sb[:, r * LCH : (r + 1) * LCH, 0:D],
                          v_out[r].rearrange("c p n -> p c n"),
                      )
                  nc.vector.memset(v_sb[:, :, D : D + 1], 1.0)

                  # Q^T (alternate DVE/ACT evictions to halve the backlog)
                  for l in range(QH):
                      for m in range(M):
                          ps = mmps.tile([P, NT], F32, tag="mm", name="ps")
                          for dc in range(DC):
                              nc.tensor.matmul(
                                  ps,
                                  wq_sb[:, dc, bass.ts(m, P)],
                                  pT[:, dc, bass.ts(l, NT)],
                                  start=(dc == 0), stop=(dc == DC - 1),
                              )
                          if m % 2 == 0:
                              nc.vector.tensor_scalar_add(
                                  qT[:, m, bass.ts(l, NT)], ps,
                                  bq_sb[:, m : m + 1],
                              )
                          else:
                              nc.scalar.activation(
                                  qT[:, m, bass.ts(l, NT)], ps, AF.Identity,
                                  bias=bq_sb[:, m : m + 1],
                              )

              # ---- phase 2: attention, S computed transposed ----
              with (
                  tc.tile_pool(name="ppool", bufs=1) as ppool,
                  tc.tile_pool(name="rpool", bufs=4) as rpool,
                  tc.tile_pool(name="opool", bufs=2) as opool,
              ):
                  p_sb = ppool.tile([P, LC, LQ_LOC], BF)  # P^T [kv, lq]
                  for lc in range(LC):
                      for qh in range(QH):
                          ps = sps.tile([P, NT], F32, tag="s", name="s")
                          if S_FP8:
                              for m in range(0, M, 2):
                                  nc.tensor.matmul(
                                      ps,
                                      kT[:, m : m + 2, bass.ts(lc, P)],
                                      qT[:, m : m + 2, bass.ts(qh, NT)],
                                      start=(m == 0), stop=(m == M - 2),
                                      perf_mode=DR,
                                  )
                          else:
                              for m in range(M):
                                  nc.tensor.matmul(
                                      ps,
                                      kT[:, m, bass.ts(lc, P)],
                                      qT[:, m, bass.ts(qh, NT)],
                                      start=(m == 0), stop=(m == M - 1),
                                  )
                          if ADD_MASK:
                              # S += -960*mask; exp((S-960m)/32) ~ 0 masked
                              nc.vector.scalar_tensor_tensor(
                                  ps, keep_sb[:, lc, bass.ts(qh, NT)],
                                  -960.0, ps, op0=ALU.mult, op1=ALU.add,
                              )
                              nc.scalar.activation(
                                  p_sb[:, lc, bass.ts(qh, NT)], ps, AF.Exp,
                                  scale=1.0 / 32.0,
                              )
                          else:
                              # P = exp(S/32) * keep (multiplicative mask)
                              nc.scalar.activation(
                                  p_sb[:, lc, bass.ts(qh, NT)], ps, AF.Exp,
                                  scale=1.0 / 32.0,
                              )
                              nc.vector.tensor_mul(
                                  p_sb[:, lc, bass.ts(qh, NT)],
                                  p_sb[:, lc, bass.ts(qh, NT)],
                                  keep_sb[:, lc, bass.ts(qh, NT)],
                              )

                  # PV (bf16): rowsum group first (ones col), then the rest
                  for qt in range(QT):
                      o_sb = opool.tile([P, D], BF, tag="o", name="o")
                      recip = rpool.tile([P, 1], F32, tag="r", name="r")
                      for gi, (off, w) in enumerate(PVG):
                          ps = avps.tile([P, 342], F32, tag="av", name="av")
                          for lc in range(LC):
                              nc.tensor.matmul(
                                  ps[:, :w],
                                  p_sb[:, lc, bass.ts(qt, P)],
                                  v_sb[:, lc, off : off + w],
                                  start=(lc == 0), stop=(lc == LC - 1),
                              )
                          if gi == 0:  # rowsum lives in the last column
                              nc.vector.reciprocal(recip, ps[:, w - 1 : w])
                              nc.scalar.activation(
                                  o_sb[:, off : off + w - 1], ps[:, : w - 1],
                                  AF.Identity, scale=recip[:, 0:1],
                              )
                          else:
                              nc.scalar.activation(
                                  o_sb[:, off : off + w], ps[:, :w],
                                  AF.Identity, scale=recip[:, 0:1],
                              )
                      nc.gpsimd.dma_start(out_d[bass.ts(qt, P), :], o_sb)

            if loop_ctx is not None:
                loop_ctx.__exit__(None, None, None)

    nc.finalize()
    return nc


def prep_in_maps(inputs: dict) -> list[dict]:
    """Host-side prep: slice per core, cast to bf16, pre-transpose."""
    bf = ml_dtypes.bfloat16
    primary = np.asarray(inputs["primary"], np.float32).astype(bf)
    ctx = np.asarray(inputs["context_sequence"], np.float32).astype(bf)
    mk = np.asarray(inputs["mask"], np.float32)
    keep = (mk if ADD_MASK else (1.0 - mk)).astype(bf)
    shared = {
        "Wq": np.asarray(inputs["Wq"], np.float32).astype(bf),
        "Wk": np.asarray(inputs["Wk"], np.float32).astype(bf),
        "Wv": np.asarray(inputs["Wv"], np.float32).astype(bf),
        "bq": np.ascontiguousarray(np.asarray(inputs["bq"], np.float32)),
        "bk": np.ascontiguousarray(np.asarray(inputs["bk"], np.float32)),
        "bv": np.asarray(inputs["bv"], np.float32).astype(bf),
    }
    H = LQ // 2
    in_maps = []
    for c in range(8):
        b, h = c // 2, c % 2
        in_maps.append(
            {
                "primaryT": np.ascontiguousarray(primary[b, h * H : (h + 1) * H, :].T),
                "contextT": np.ascontiguousarray(ctx[b, h * H : (h + 1) * H, :].T),
                "keepT": np.ascontiguousarray(keep[b, h * H : (h + 1) * H, :].T),
                **shared,
            }
        )
    return in_maps


_NC_CACHE = None


def kernel(**inputs: np.ndarray) -> np.ndarray:
    global _NC_CACHE
    if _NC_CACHE is None:
        _NC_CACHE = build_nc()
    nc = _NC_CACHE

    in_maps = prep_in_maps(inputs)
    res = bass_utils.run_bass_kernel_spmd(nc, in_maps, core_ids=list(range(8)))

    H = LQ // 2
    out = np.empty((B, LQ, D), dtype=np.float32)
    for c in range(8):
        b, h = c // 2, c % 2
        out[b, h * H : (h + 1) * H, :] = res.results[c]["out"].astype(np.float32)
    return out


if __name__ == "__main__":
    rng = np.random.default_rng(0)
    ins = {
        "primary": rng.standard_normal((B, LQ, D), dtype=np.float32),
        "context_sequence": rng.standard_normal((B, LKV, D), dtype=np.float32),
        "mask": rng.integers(0, 2, (B, LQ, LKV)).astype(np.float32),
        "Wq": rng.uniform(-1 / 32, 1 / 32, (D, D)).astype(np.float32),
        "bq": rng.uniform(-1 / 32, 1 / 32, (D,)).astype(np.float32),
        "Wk": rng.uniform(-1 / 32, 1 / 32, (D, D)).astype(np.float32),
        "bk": rng.uniform(-1 / 32, 1 / 32, (D,)).astype(np.float32),
        "Wv": rng.uniform(-1 / 32, 1 / 32, (D, D)).astype(np.float32),
        "bv": rng.uniform(-1 / 32, 1 / 32, (D,)).astype(np.float32),
    }
    out = kernel(**ins)
    print("out", out.shape, out.dtype, float(np.abs(out).mean()))


# revision 23
# speedup vs baseline: 1.0746x; 1.0746x over previous
"""Fused attention kernel for Trainium2, SPMD over 8 NeuronCores.

Problem: nn_Attention_2808908611625
  q = primary @ Wq + bq;  k = ctx @ Wk + bk;  v = ctx @ Wv + bv
  out = softmax(q k^T / sqrt(1024) - 1e9 * mask) @ v

Sharding: core c handles batch b = c//2, query-row half h = c%2
  (1024 query rows per core, full K/V context of its batch; K/V projection
  split across the core pair and exchanged with a pair AllGather).

Precision plan (PE matmul cycles are the roofline; measured on HW):
  * Projections and PV run in bf16 (1 cyc/output-row; fp8 DoubleRow's ~2x
    only helps if ONE term suffices -- residual-split needs 3 terms and
    loses, plain-fp8 x/W or P/V fails the 2e-2 gate at ~3e-2).
  * S^T = k q runs in PLAIN fp8 DoubleRow at free-dim 512 (the profitable
    regime: 2 contraction k-tiles per instruction, measured ~1.9x bf16).
    q, k are quantized to fp8e4 at their projection PSUM evictions (free).
    This is the dominant error term: ~1.2e-2 output rel err, inside the
    gate, and S is the only matmul where one fp8 term is enough.

Layout/schedule:
  * Host pre-transposes everything so every DMA lands contraction-major;
    no TensorE transposes. ctx/Wk input DMAs are chunked (dc-pairs) and
    spread over the SP + ACT queues so the first K-proj matmul starts
    ~2us in instead of waiting for whole tensors.
  * S is computed TRANSPOSED: S^T[kv, q] = sum_m kT.T @ qT, so the exp
    eviction writes P^T in exactly the [kv, q] layout PV needs stationary.
  * Mask is applied MULTIPLICATIVELY: host ships keep = 1-mask (bf16);
    ACT evicts P^T = exp(S^T/32) (PSUM->SBUF bf16), then one in-place DVE
    multiply by keep zeroes the masked entries (exact; 16-bit DVE 2x).
    No max-subtraction: |S/32| <= ~2.5 so exp <= ~12, bf16-safe. The whole
    keep matrix is prefetched during phase 1 as one 2MB DMA.
  * The k pair-exchange (write + AllGather + readback) is issued before
    the v exchange on the gpsimd queue so kT is resident long before the
    S matmuls (no phase-boundary PE gap). Biases fold into PSUM evictions
    (ACT Identity + bias for k and half of q, DVE tensor_scalar for the
    other half, DVE scalar_tensor_tensor for v's per-column bv).
  * V carries a 1025th column of ones; PV's 1025 output columns split into
    PSUM groups of 341/342/342 (rowsum group first) so the softmax row-sum
    falls out of the ones column, evicted with a per-partition 1/rowsum
    ACT scale. Output bf16, host upcasts.
"""

import numpy as np
import ml_dtypes

import concourse.bass as bass
import concourse.mybir as mybir
import concourse.tile as tile
from concourse import bacc, bass_utils

BF = mybir.dt.bfloat16
F32 = mybir.dt.float32
FP8 = mybir.dt.float8e4
AF = mybir.ActivationFunctionType
ALU = mybir.AluOpType
DR = mybir.MatmulPerfMode.DoubleRow

B, LQ, LKV, D = 4, 2048, 2048, 1024
P = 128
LQ_LOC = (B * LQ) // 8  # 1024 query rows per core
DC = D // P             # 8 contraction chunks
M = D // P              # 8 attn-dim chunks
NT = 512                # psum tile width
HKV = LKV // 2          # per-core K/V rows (pair-sharded)
LTH = HKV // NT         # 2 own kv column tiles (K^T layout)
LCH = HKV // P          # 8 own kv row chunks (V layout)
LC = LKV // P           # 16 kv chunks total
QT = LQ_LOC // P        # 8 q row tiles per core
QH = LQ_LOC // NT       # 2 q halves for S^T
PVG = [(684, 341), (0, 342), (342, 342)]  # PV psum groups; rowsum group first
VW = D + 1              # v width incl. ones col at D

UNROLL_REPS = False
STUB_CC = False   # force the local-DMA collective stub even at reps=1 (sim)
S_FP8 = True      # S^T matmuls in fp8 DoubleRow (else bf16)
ADD_MASK = False  # additive -960*mask on PSUM (else multiplicative keep)
LOADS_BASELINE = True  # monolithic loads, all on the SP queue
CHUNK_SYNC = True  # chunked cT/wk, everything on the SP queue (overrides)


def build_nc(reps: int = 1):
    nc = bacc.Bacc("TRN2", num_swdge_queues=4, num_devices=8)

    pT_d = nc.dram_tensor("primaryT", (D, LQ_LOC), BF, kind="ExternalInput")
    cT_d = nc.dram_tensor("contextT", (D, HKV), BF, kind="ExternalInput")
    keepT_d = nc.dram_tensor("keepT", (LKV, LQ_LOC), BF, kind="ExternalInput")
    wq_d = nc.dram_tensor("Wq", (D, D), BF, kind="ExternalInput")
    wk_d = nc.dram_tensor("Wk", (D, D), BF, kind="ExternalInput")
    wv_d = nc.dram_tensor("Wv", (D, D), BF, kind="ExternalInput")
    bq_d = nc.dram_tensor("bq", (D,), F32, kind="ExternalInput")
    bk_d = nc.dram_tensor("bk", (D,), F32, kind="ExternalInput")
    bv_d = nc.dram_tensor("bv", (D,), BF, kind="ExternalInput")
    out_d = nc.dram_tensor("out", (LQ_LOC, D), BF, kind="ExternalOutput")

    with tile.TileContext(nc) as tc:
        with (
            tc.tile_pool(name="const", bufs=1) as const,
            tc.tile_pool(name="persist", bufs=1) as persist,
            tc.tile_pool(name="dram", bufs=1, space="DRAM") as dram,
            tc.tile_pool(name="mmps", bufs=3, space="PSUM") as mmps,
            tc.tile_pool(name="sps", bufs=3, space="PSUM") as sps,
            tc.tile_pool(name="avps", bufs=2, space="PSUM") as avps,
        ):
            # biases: b*_sb[p, m] = b[m*128 + p] (descriptor-heavy; ACT queue)
            bq_sb = const.tile([P, M], F32)
            bk_sb = const.tile([P, M], F32)
            with nc.allow_non_contiguous_dma(reason="tiny bias vectors"):
                nc.scalar.dma_start(bq_sb, bq_d[:].rearrange("(m p) -> p m", p=P))
                nc.scalar.dma_start(bk_sb, bk_d[:].rearrange("(m p) -> p m", p=P))

            # bv broadcast to all partitions: ones[1,128].T @ bv[1, D]
            bv_row = const.tile([1, D], BF)
            nc.scalar.dma_start(bv_row, bv_d[:].rearrange("(one n) -> one n", one=1))
            ones_row = const.tile([1, P], BF)
            nc.vector.memset(ones_row, 1.0)
            bv_bcast = const.tile([P, D], F32)

            qkd = FP8 if S_FP8 else BF
            qT = persist.tile([P, M, LQ_LOC], qkd)   # q^T   [dattn, lq]
            kT = persist.tile([P, M, LKV], qkd)      # k^T   [dattn, lkv]
            v_sb = persist.tile([P, LC, VW], BF)     # v+bv  [lkv, dout | 1]
            keep_sb = persist.tile([P, LC, LQ_LOC], BF)  # 1-mask [kv, lq]

            # pair exchange buffers (AllGather within core pairs). Own halves
            # are evicted in place into kT/v_sb; the gathered copy is read
            # back in global kv order.
            k_in = dram.tile([M, P, HKV], qkd, name="k_in")
            k_out = dram.tile([2, M, P, HKV], qkd, name="k_out")
            v_in = dram.tile([LCH, P, D], BF, name="v_in")
            v_out = dram.tile([2, LCH, P, D], BF, name="v_out")
            RG = [[0, 1], [2, 3], [4, 5], [6, 7]]

            # bv broadcast is constant across reps -- compute once
            for n in range(D // NT):
                ps = mmps.tile([P, NT], F32, tag="mm", name="ps")
                nc.tensor.matmul(
                    ps, ones_row, bv_row[:, bass.ts(n, NT)],
                    start=True, stop=True,
                )
                nc.scalar.activation(bv_bcast[:, bass.ts(n, NT)], ps, AF.Copy)

            collective_in_body = (reps == 1 or UNROLL_REPS) and not STUB_CC
            loop_ctx = None
            if reps > 1 and not UNROLL_REPS:
                loop_ctx = tc.For_i(0, reps, 1)
                loop_ctx.__enter__()

            for _rep in range(reps if UNROLL_REPS else 1):
              # ---- phase 1: loads + K/V/Q projections + pair exchange ----
              with (
                  tc.tile_pool(name="w", bufs=1) as wp,
                  tc.tile_pool(name="xT", bufs=1) as xtp,
              ):
                  wq_sb = wp.tile([P, DC, D], BF)
                  wk_sb = wp.tile([P, DC, D], BF)
                  wv_sb = wp.tile([P, DC, D], BF)
                  pT = xtp.tile([P, DC, LQ_LOC], BF)  # primary^T [din, lq]
                  cT = xtp.tile([P, DC, HKV], BF)     # ctx^T [din, own half]

                  # SP queue: ctx + Wk interleaved in dc-pair chunks (K-proj
                  # starts after ~1MB), then primary. ACT queue: Wv, Wq, keep.
                  def chunk_load(eng, t, d, dp):
                      eng.dma_start(
                          t[:, 2 * dp : 2 * dp + 2, :],
                          d[2 * dp * P : (2 * dp + 2) * P, :].rearrange(
                              "(dc p) n -> p dc n", p=P
                          ),
                      )

                  if CHUNK_SYNC:
                      for dp in range(DC // 2):
                          chunk_load(nc.sync, cT, cT_d, dp)
                          chunk_load(nc.sync, wk_sb, wk_d, dp)
                      for t, d in ((wv_sb, wv_d), (pT, pT_d), (wq_sb, wq_d)):
                          nc.sync.dma_start(
                              t, d[:].rearrange("(dc p) n -> p dc n", p=P)
                          )
                      nc.sync.dma_start(
                          keep_sb,
                          keepT_d[:].rearrange("(lc p) n -> p lc n", p=P),
                      )
                  elif LOADS_BASELINE:
                      nc.sync.dma_start(
                          cT, cT_d[:].rearrange("(dc p) n -> p dc n", p=P)
                      )
                      for h in range(2):
                          HW2 = D // 2
                          nc.sync.dma_start(
                              wk_sb[:, :, h * HW2 : (h + 1) * HW2],
                              wk_d[:, h * HW2 : (h + 1) * HW2].rearrange(
                                  "(dc p) n -> p dc n", p=P
                              ),
                          )
                      nc.sync.dma_start(
                          wv_sb, wv_d[:].rearrange("(dc p) n -> p dc n", p=P)
                      )
                      nc.sync.dma_start(
                          pT, pT_d[:].rearrange("(dc p) n -> p dc n", p=P)
                      )
                      nc.sync.dma_start(
                          wq_sb, wq_d[:].rearrange("(dc p) n -> p dc n", p=P)
                      )
                      nc.sync.dma_start(
                          keep_sb,
                          keepT_d[:].rearrange("(lc p) n -> p lc n", p=P),
                      )
                  else:
                      for dp in range(DC // 2):
                          chunk_load(nc.sync, cT, cT_d, dp)
                          chunk_load(nc.sync, wk_sb, wk_d, dp)
                      nc.scalar.dma_start(
                          wv_sb, wv_d[:].rearrange("(dc p) n -> p dc n", p=P)
                      )
                      nc.sync.dma_start(
                          pT, pT_d[:].rearrange("(dc p) n -> p dc n", p=P)
                      )
                      nc.scalar.dma_start(
                          wq_sb, wq_d[:].rearrange("(dc p) n -> p dc n", p=P)
                      )
                      nc.scalar.dma_start(
                          keep_sb,
                          keepT_d[:].rearrange("(lc p) n -> p lc n", p=P),
                      )

                  # K^T own half -> kT half-0 slot (ACT folds bk, fp8 out);
                  # the post-collective readback rewrites kT in global order.
                  for l in range(LTH):
                      for m in range(M):
                          ps = mmps.tile([P, NT], F32, tag="mm", name="ps")
                          for dc in range(DC):
                              nc.tensor.matmul(
                                  ps,
                                  wk_sb[:, dc, bass.ts(m, P)],
                                  cT[:, dc, bass.ts(l, NT)],
                                  start=(dc == 0), stop=(dc == DC - 1),
                              )
                          nc.scalar.activation(
                              kT[:, m, bass.ts(l, NT)], ps, AF.Identity,
                              bias=bk_sb[:, m : m + 1],
                          )
                  # full k exchange chain first: kT must be resident for S
                  nc.gpsimd.dma_start(
                      k_in[:].rearrange("m p h -> p m h"), kT[:, :, 0:HKV]
                  )
                  if collective_in_body:
                      nc.gpsimd.collective_compute(
                          "AllGather", ALU.bypass, replica_groups=RG,
                          ins=[k_in[:]], outs=[k_out[:]],
                      )
                  else:  # timing stub: same bytes moved, no cross-core sync
                      for r in range(2):
                          nc.gpsimd.dma_start(
                              k_out[r].rearrange("m p h -> p m h"),
                              kT[:, :, 0:HKV],
                          )
                  for r in range(2):
                      nc.gpsimd.dma_start(
                          kT[:, :, r * HKV : (r + 1) * HKV],
                          k_out[r].rearrange("m p h -> p m h"),
                      )

                  # V own half (+bv via one DVE stt) -> v_sb chunk-0 slot
                  for lc in range(LCH):
                      for n in range(D // NT):
                          ps = mmps.tile([P, NT], F32, tag="mm", name="ps")
                          for dc in range(DC):
                              nc.tensor.matmul(
                                  ps,
                                  cT[:, dc, bass.ts(lc, P)],
                                  wv_sb[:, dc, bass.ts(n, NT)],
                                  start=(dc == 0), stop=(dc == DC - 1),
                              )
                          nc.vector.scalar_tensor_tensor(
                              v_sb[:, lc, bass.ts(n, NT)], ps, 1.0,
                              bv_bcast[:, bass.ts(n, NT)],
                              op0=ALU.mult, op1=ALU.add,
                          )
                  nc.gpsimd.dma_start(
                      v_in[:].rearrange("c p n -> p c n"), v_sb[:, 0:LCH, 0:D]
                  )
                  if collective_in_body:
                      nc.gpsimd.collective_compute(
                          "AllGather", ALU.bypass, replica_groups=RG,
                          ins=[v_in[:]], outs=[v_out[:]],
                      )
                  else:  # timing stub
                      for r in range(2):
                          nc.gpsimd.dma_start(
                              v_out[r].rearrange("c p n -> p c n"),
                              v_sb[:, 0:LCH, 0:D],
                          )
                  for r in range(2):
                      nc.gpsimd.dma_start(
                          v_sb[:, r * LCH : (r + 1) * LCH, 0:D],
                          v_out[r].rearrange("c p n -> p c n"),
                      )
                  nc.vector.memset(v_sb[:, :, D : D + 1], 1.0)

                  # Q^T (alternate DVE/ACT evictions to halve the backlog)
                  for l in range(QH):
                      for m in range(M):
                          ps = mmps.tile([P, NT], F32, tag="mm", name="ps")
                          for dc in range(DC):
                              nc.tensor.matmul(
                                  ps,
                                  wq_sb[:, dc, bass.ts(m, P)],
                                  pT[:, dc, bass.ts(l, NT)],
                                  start=(dc == 0), stop=(dc == DC - 1),
                              )
                          if m % 2 == 0:
                              nc.vector.tensor_scalar_add(
                                  qT[:, m, bass.ts(l, NT)], ps,
                                  bq_sb[:, m : m + 1],
                              )
                          else:
                              nc.scalar.activation(
                                  qT[:, m, bass.ts(l, NT)], ps, AF.Identity,
                                  bias=bq_sb[:, m : m + 1],
                              )

              # ---- phase 2: attention, S computed transposed ----
              with (
                  tc.tile_pool(name="ppool", bufs=1) as ppool,
                  tc.tile_pool(name="rpool", bufs=4) as rpool,
                  tc.tile_pool(name="opool", bufs=2) as opool,
              ):
                  p_sb = ppool.tile([P, LC, LQ_LOC], BF)  # P^T [kv, lq]
                  for lc in range(LC):
                      for qh in range(QH):
                          ps = sps.tile([P, NT], F32, tag="s", name="s")
                          if S_FP8:
                              for m in range(0, M, 2):
                                  nc.tensor.matmul(
                                      ps,
                                      kT[:, m : m + 2, bass.ts(lc, P)],
                                      qT[:, m : m + 2, bass.ts(qh, NT)],
                                      start=(m == 0), stop=(m == M - 2),
                                      perf_mode=DR,
                                  )
                          else:
                              for m in range(M):
                                  nc.tensor.matmul(
                                      ps,
                                      kT[:, m, bass.ts(lc, P)],
                                      qT[:, m, bass.ts(qh, NT)],
                                      start=(m == 0), stop=(m == M - 1),
                                  )
                          if ADD_MASK:
                              # S += -960*mask; exp((S-960m)/32) ~ 0 masked
                              nc.vector.scalar_tensor_tensor(
                                  ps, keep_sb[:, lc, bass.ts(qh, NT)],
                                  -960.0, ps, op0=ALU.mult, op1=ALU.add,
                              )
                              nc.scalar.activation(
                                  p_sb[:, lc, bass.ts(qh, NT)], ps, AF.Exp,
                                  scale=1.0 / 32.0,
                              )
                          else:
                              # P = exp(S/32) * keep (multiplicative mask)
                              nc.scalar.activation(
                                  p_sb[:, lc, bass.ts(qh, NT)], ps, AF.Exp,
                                  scale=1.0 / 32.0,
                              )
                              nc.vector.tensor_mul(
                                  p_sb[:, lc, bass.ts(qh, NT)],
                                  p_sb[:, lc, bass.ts(qh, NT)],
                                  keep_sb[:, lc, bass.ts(qh, NT)],
                              )

                  # PV (bf16): rowsum group first (ones col), then the rest
                  for qt in range(QT):
                      o_sb = opool.tile([P, D], BF, tag="o", name="o")
                      recip = rpool.tile([P, 1], F32, tag="r", name="r")
                      for gi, (off, w) in enumerate(PVG):
                          ps = avps.tile([P, 342], F32, tag="av", name="av")
                          for lc in range(LC):
                              nc.tensor.matmul(
                                  ps[:, :w],
                                  p_sb[:, lc, bass.ts(qt, P)],
                                  v_sb[:, lc, off : off + w],
                                  start=(lc == 0), stop=(lc == LC - 1),
                              )
                          if gi == 0:  # rowsum lives in the last column
                              nc.vector.reciprocal(recip, ps[:, w - 1 : w])
                              nc.scalar.activation(
                                  o_sb[:, off : off + w - 1], ps[:, : w - 1],
                                  AF.Identity, scale=recip[:, 0:1],
                              )
                          else:
                              nc.scalar.activation(
                                  o_sb[:, off : off + w], ps[:, :w],
                                  AF.Identity, scale=recip[:, 0:1],
                              )
                      nc.gpsimd.dma_start(out_d[bass.ts(qt, P), :], o_sb)

            if loop_ctx is not None:
                loop_ctx.__exit__(None, None, None)

    nc.finalize()
    return nc


def prep_in_maps(inputs: dict) -> list[dict]:
    """Host-side prep: slice per core, cast to bf16, pre-transpose."""
    bf = ml_dtypes.bfloat16
    primary = np.asarray(inputs["primary"], np.float32).astype(bf)
    ctx = np.asarray(inputs["context_sequence"], np.float32).astype(bf)
    mk = np.asarray(inputs["mask"], np.float32)
    keep = (mk if ADD_MASK else (1.0 - mk)).astype(bf)
    shared = {
        "Wq": np.asarray(inputs["Wq"], np.float32).astype(bf),
        "Wk": np.asarray(inputs["Wk"], np.float32).astype(bf),
        "Wv": np.asarray(inputs["Wv"], np.float32).astype(bf),
        "bq": np.ascontiguousarray(np.asarray(inputs["bq"], np.float32)),
        "bk": np.ascontiguousarray(np.asarray(inputs["bk"], np.float32)),
        "bv": np.asarray(inputs["bv"], np.float32).astype(bf),
    }
    H = LQ // 2
    in_maps = []
    for c in range(8):
        b, h = c // 2, c % 2
        in_maps.append(
            {
                "primaryT": np.ascontiguousarray(primary[b, h * H : (h + 1) * H, :].T),
                "contextT": np.ascontiguousarray(ctx[b, h * H : (h + 1) * H, :].T),
                "keepT": np.ascontiguousarray(keep[b, h * H : (h + 1) * H, :].T),
                **shared,
            }
        )
    return in_maps


_NC_CACHE = None


def kernel(**inputs: np.ndarray) -> np.ndarray:
    global _NC_CACHE
    if _NC_CACHE is None:
        _NC_CACHE = build_nc()
    nc = _NC_CACHE

    in_maps = prep_in_maps(inputs)
    res = bass_utils.run_bass_kernel_spmd(nc, in_maps, core_ids=list(range(8)))

    H = LQ // 2
    out = np.empty((B, LQ, D), dtype=np.float32)
    for c in range(8):
        b, h = c // 2, c % 2
        out[b, h * H : (h + 1) * H, :] = res.results[c]["out"].astype(np.float32)
    return out


if __name__ == "__main__":
    rng = np.random.default_rng(0)
    ins = {
        "primary": rng.standard_normal((B, LQ, D), dtype=np.float32),
        "context_sequence": rng.standard_normal((B, LKV, D), dtype=np.float32),
        "mask": rng.integers(0, 2, (B, LQ, LKV)).astype(np.float32),
        "Wq": rng.uniform(-1 / 32, 1 / 32, (D, D)).astype(np.float32),
        "bq": rng.uniform(-1 / 32, 1 / 32, (D,)).astype(np.float32),
        "Wk": rng.uniform(-1 / 32, 1 / 32, (D, D)).astype(np.float32),
        "bk": rng.uniform(-1 / 32, 1 / 32, (D,)).astype(np.float32),
        "Wv": rng.uniform(-1 / 32, 1 / 32, (D, D)).astype(np.float32),
        "bv": rng.uniform(-1 / 32, 1 / 32, (D,)).astype(np.float32),
    }
    out = kernel(**ins)
    print("out", out.shape, out.dtype, float(np.abs(out).mean()))
